# revision 1
# baseline (speedup 1.0000x reference)
"""Centroid triplet loss on 8 Trainium2 NeuronCores (Bass/Tile).

Data-parallel over the batch: each of the 8 cores gets 8192 of the 65536
samples.  Per-class embedding sums and counts are all-reduced to form global
centroids; each core then computes its local triplet terms and a final
all-reduce produces the scalar loss.

Math restructure (equivalent to the reference):
    term_i = relu(margin + e_hat_i . (cent[nearest[l_i]] - cent[l_i]))
    loss   = sum_i w_{l_i} * term_i / n_present,   w_c = 1/max(count_c, 1)
Since relu(w*x) = w*relu(x) for w > 0, a sample's weighted term is
    relu(b_{l_i} + r_i * (e_i . u_{l_i}))
with u_c = w_c*(cent_near_c - cent_c), b_c = w_c*margin, r_i = 1/||e_i||.
So embeddings stay raw in SBUF; the one-hot used for the class-sum matmul is
scaled by r_i, and pass 2 gathers (u_c, b_c) rows per sample by label and
fuses the dot product via tensor_tensor_reduce.
"""

import sys

for _p in ("/opt/trn_rl_repo",):
    if _p not in sys.path:
        sys.path.insert(0, _p)

from contextlib import ExitStack

import numpy as np

from concourse import bacc, bass, mybir, tile
from concourse.bass_utils import run_bass_kernel_spmd
from concourse.masks import make_identity

F32 = mybir.dt.float32
BF16 = mybir.dt.bfloat16
I32 = mybir.dt.int32
I16 = mybir.dt.int16
ALU = mybir.AluOpType
ACTF = mybir.ActivationFunctionType

N_CORES = 8
B_FULL = 65536
D = 512
C = 256
MARGIN = 0.3
EPS = 1e-12

P = 128                      # SBUF partitions
B_LOC = B_FULL // N_CORES    # 8192 samples per core
T = B_LOC // P               # 64 sample tiles of 128
LOAD_CHUNK = 8               # tiles per embedding-load DMA (2 MiB each)
TBL_B = 640                  # bf16 table row: k*u[0:512], b_hi, b_lo, u2_hi, u2_lo, pad
                             # (1280B, multiple of 256B for dma_gather)
GCHUNK = 1024                # indices per dma_gather call (8 sample tiles)
NEG = -1e30
KAPPA = 256.0                # scale for the difference-of-squares dot trick


def _build():
    nc = bacc.Bacc(
        "TRN2",
        target_bir_lowering=False,
        debug=False,
        enable_asserts=False,
        num_devices=N_CORES,
    )

    emb = nc.dram_tensor("emb", [B_LOC, D], F32, kind="ExternalInput")
    lab = nc.dram_tensor("lab", [P, T], I32, kind="ExternalInput")
    # labels in dma_gather's wrapped-int16 layout: idx i lives at
    # [i % 16, i // 16], replicated into all eight 16-partition groups
    lab16 = nc.dram_tensor("lab16", [P, B_LOC // 16], I16, kind="ExternalInput")
    loss_out = nc.dram_tensor("loss", [1, 1], F32, kind="ExternalOutput")

    # Internal HBM scratch.  AR1 buffer: rows 0:256 per-class sums, row 256
    # carries the per-class counts in its first 256 columns.
    ar1_in = nc.dram_tensor("ar1_in", [C + 1, D], F32)
    ar1_out = nc.dram_tensor("ar1_out", [C + 1, D], F32, addr_space="Shared")
    table = nc.dram_tensor("tbl", [C, TBL_B], BF16)
    ar2_in = nc.dram_tensor("ar2_in", [1, 8], F32)
    ar2_out = nc.dram_tensor("ar2_out", [1, 8], F32, addr_space="Shared")

    groups = [list(range(N_CORES))]

    with tile.TileContext(nc) as tc, ExitStack() as ctx:
        const = ctx.enter_context(tc.tile_pool(name="const", bufs=1))
        big = ctx.enter_context(tc.tile_pool(name="big", bufs=1))
        work = ctx.enter_context(tc.tile_pool(name="work", bufs=3))
        sq = ctx.enter_context(tc.tile_pool(name="sq", bufs=2))
        gat = ctx.enter_context(tc.tile_pool(name="gat", bufs=4))
        mid = ctx.enter_context(tc.tile_pool(name="mid", bufs=1))
        psacc = ctx.enter_context(tc.tile_pool(name="psacc", bufs=1, space="PSUM"))
        psmid = ctx.enter_context(tc.tile_pool(name="psmid", bufs=3, space="PSUM"))

        # ---- constants -------------------------------------------------
        ident = const.tile([P, P], F32)
        make_identity(nc, ident[:])
        iota_row = const.tile([P, C], BF16)
        nc.gpsimd.iota(
            iota_row[:], pattern=[[1, C]], base=0, channel_multiplier=0,
            allow_small_or_imprecise_dtypes=True,
        )
        ones_col = const.tile([P, 1], F32)
        nc.gpsimd.memset(ones_col[:], 1.0)
        ones_col_bf = const.tile([P, 1], BF16)
        nc.gpsimd.memset(ones_col_bf[:], 1.0)
        ones_row = const.tile([1, P], F32)
        nc.gpsimd.memset(ones_row[:], 1.0)

        lab_sb = const.tile([P, T], I32)
        nc.sync.dma_start(out=lab_sb[:], in_=lab.ap())
        lab_f = const.tile([P, T], BF16)
        nc.vector.tensor_copy(out=lab_f[:], in_=lab_sb[:])
        lab16_sb = const.tile([P, B_LOC // 16], I16)
        nc.sync.dma_start(out=lab16_sb[:], in_=lab16.ap())

        # ---- pass 1: load embeddings, norms, class sums/counts ---------
        e_chunks = []
        emb_v = emb.ap().rearrange("(t p) d -> p t d", p=P)
        for ci in range(T // LOAD_CHUNK):
            # bf16 residency: halves SBUF and lets the class-sum matmuls run
            # single-pass bf16 instead of fp32 HI/LO pairs (cast in the DMA,
            # SWDGE-only feature)
            ec = big.tile([P, LOAD_CHUNK, D], BF16, tag=f"e{ci}")
            e_chunks.append(ec)
            sl = slice(ci * LOAD_CHUNK, (ci + 1) * LOAD_CHUNK)
            nc.gpsimd.dma_start(out=ec[:], in_=emb_v[:, sl, :])

        norm2 = const.tile([P, T], F32)
        norm = const.tile([P, T], F32)
        r_all = const.tile([P, T], F32)
        r_bf = const.tile([P, T], BF16)

        sums_ps0 = psacc.tile([P, D], F32, tag="sums0")
        sums_ps1 = psacc.tile([P, D], F32, tag="sums1")
        cnt_ps = psacc.tile([1, C], F32, tag="cnt")

        def e_tile(t):
            return e_chunks[t // LOAD_CHUNK][:, t % LOAD_CHUNK, :]

        for ci in range(T // LOAD_CHUNK):
            csl = slice(ci * LOAD_CHUNK, (ci + 1) * LOAD_CHUNK)
            for j in range(LOAD_CHUNK):
                t = ci * LOAD_CHUNK + j
                sq_t = sq.tile([P, D], F32, tag="sq")
                # tensor_tensor_reduce is broken on this runtime (kills the
                # exec unit) — use ACT Square with free-dim accumulation.
                nc.scalar.activation(
                    sq_t[:], e_tile(t), ACTF.Square,
                    accum_out=norm2[:, t : t + 1],
                )
            # batched per-chunk norm -> r (cheaper than per-tile column ops)
            nc.scalar.activation(norm[:, csl], norm2[:, csl], ACTF.Sqrt)
            nc.vector.reciprocal(r_all[:, csl], norm[:, csl])
            nc.vector.tensor_copy(out=r_bf[:, csl], in_=r_all[:, csl])

            for j in range(LOAD_CHUNK):
                t = ci * LOAD_CHUNK + j
                et = e_tile(t)
                # plain one-hot (tensor_scalar is ~10x slower than broadcast
                # tensor_tensor — use TT against a bf16 iota)
                oht = work.tile([P, C], BF16, tag="oht")
                nc.vector.tensor_tensor(
                    out=oht[:], in0=iota_row[:],
                    in1=lab_f[:, t : t + 1].to_broadcast([P, C]),
                    op=ALU.is_equal,
                )
                # r-scaled one-hot for the normalized class sums; alternate
                # the scaling between ACT and DVE to balance engine load
                osc = work.tile([P, C], BF16, tag="osc")
                if t % 2 == 0:
                    nc.scalar.activation(
                        osc[:], oht[:], ACTF.Copy, scale=r_all[:, t : t + 1]
                    )
                else:
                    nc.vector.tensor_tensor(
                        out=osc[:], in0=oht[:],
                        in1=r_bf[:, t : t + 1].to_broadcast([P, C]),
                        op=ALU.mult,
                    )
                first, last = t == 0, t == T - 1
                nc.tensor.matmul(
                    sums_ps0[:], osc[:, 0:P], et, start=first, stop=last
                )
                nc.tensor.matmul(
                    sums_ps1[:], osc[:, P:C], et, start=first, stop=last
                )
                nc.tensor.matmul(
                    cnt_ps[:], ones_col_bf[:], oht[:], start=first, stop=last
                )

        # ---- all-reduce sums + counts ----------------------------------
        sums_sb = [mid.tile([P, D], F32, tag=f"ssb{h}", name=f"ssb{h}") for h in range(2)]
        nc.vector.tensor_copy(out=sums_sb[0][:], in_=sums_ps0[:])
        nc.vector.tensor_copy(out=sums_sb[1][:], in_=sums_ps1[:])
        cnt_row = mid.tile([1, D], F32, tag="cntrow")
        nc.vector.memset(cnt_row[:], 0.0)
        nc.vector.tensor_copy(out=cnt_row[:, 0:C], in_=cnt_ps[:])

        nc.sync.dma_start(out=ar1_in.ap()[0:P, :], in_=sums_sb[0][:])
        nc.sync.dma_start(out=ar1_in.ap()[P:C, :], in_=sums_sb[1][:])
        nc.sync.dma_start(out=ar1_in.ap()[C : C + 1, :], in_=cnt_row[:])

        nc.gpsimd.collective_compute(
            "AllReduce", ALU.add, replica_groups=groups,
            ins=[ar1_in.ap()], outs=[ar1_out.ap()],
        )

        # global sums overwrite the local-sum tiles (same slots, AR is done)
        gsums = [mid.tile([P, D], F32, tag=f"ssb{h}", name=f"gs{h}") for h in range(2)]
        nc.sync.dma_start(out=gsums[0][:], in_=ar1_out.ap()[0:P, :])
        nc.sync.dma_start(out=gsums[1][:], in_=ar1_out.ap()[P:C, :])
        gcnt_row = mid.tile([1, C], F32, tag="cntrow")
        nc.sync.dma_start(out=gcnt_row[:], in_=ar1_out.ap()[C : C + 1, 0:C])

        # ---- centroids: cent = sums / max(||sums||, eps) ---------------
        cent = []
        for h in range(2):
            s2 = sq.tile([P, D], F32, tag="sq")  # scratch for the squares
            cn2 = mid.tile([P, 1], F32, tag=f"cn{h}")
            nc.scalar.activation(
                s2[:], gsums[h][:], ACTF.Square, accum_out=cn2[:]
            )
            nc.scalar.activation(cn2[:], cn2[:], ACTF.Sqrt)
            nc.vector.tensor_scalar(
                out=cn2[:], in0=cn2[:], scalar1=EPS, scalar2=None, op0=ALU.max
            )
            nc.vector.reciprocal(cn2[:], cn2[:])
            ch = mid.tile([P, D], F32, tag=f"cent{h}")
            nc.vector.tensor_scalar(
                out=ch[:], in0=gsums[h][:], scalar1=cn2[:], scalar2=None,
                op0=ALU.mult,
            )
            cent.append(ch)

        # ---- presence masks, counts columns, w -------------------------
        negmask_r = mid.tile([1, C], F32, tag="negm")
        nc.vector.tensor_scalar(
            out=negmask_r[:], in0=gcnt_row[:], scalar1=0.5, scalar2=float(NEG),
            op0=ALU.is_lt, op1=ALU.mult,
        )
        present_r = mid.tile([1, C], F32, tag="pres")
        nc.vector.tensor_scalar(
            out=present_r[:], in0=gcnt_row[:], scalar1=0.5, scalar2=None,
            op0=ALU.is_ge,
        )
        npres = mid.tile([1, 1], F32, tag="npres")
        nc.vector.reduce_sum(npres[:], present_r[:], axis=mybir.AxisListType.X)
        nc.vector.tensor_scalar(
            out=npres[:], in0=npres[:], scalar1=1.0, scalar2=None, op0=ALU.max
        )
        inv_np = mid.tile([1, 1], F32, tag="invnp")
        nc.vector.reciprocal(inv_np[:], npres[:])

        wcol = []
        for h in range(2):
            ccol_ps = psmid.tile([P, 1], F32, tag="m")
            nc.tensor.matmul(
                ccol_ps[:], gcnt_row[:, h * P : (h + 1) * P], ones_row[:, 0:1]
            )
            wc = mid.tile([P, 1], F32, tag=f"w{h}")
            nc.vector.tensor_scalar(
                out=wc[:], in0=ccol_ps[:], scalar1=1.0, scalar2=None, op0=ALU.max
            )
            nc.vector.reciprocal(wc[:], wc[:])
            wcol.append(wc)

        # ---- centroid similarity G = cent @ cent.T ---------------------
        centT = [mid.tile([P, C], F32, tag=f"ct{k}", name=f"ct{k}") for k in range(4)]
        for h in range(2):
            for k in range(4):
                tp = psmid.tile([P, P], F32, tag="m")
                nc.tensor.transpose(
                    tp[:], cent[h][:, k * P : (k + 1) * P], ident[:]
                )
                nc.vector.tensor_copy(
                    out=centT[k][:, h * P : (h + 1) * P], in_=tp[:]
                )

        g_sb = []
        for h in range(2):
            gp = psmid.tile([P, C], F32, tag="m")
            for k in range(4):
                nc.tensor.matmul(
                    gp[:], centT[k][:, h * P : (h + 1) * P], centT[k][:],
                    start=(k == 0), stop=(k == 3),
                )
            gs = mid.tile([P, C], F32, tag=f"g{h}")
            nc.vector.tensor_copy(out=gs[:], in_=gp[:])
            # mask the diagonal (self-similarity): keep where col - row != 0
            nc.gpsimd.affine_select(
                out=gs[:], in_=gs[:], compare_op=ALU.not_equal, fill=NEG,
                base=-h * P, pattern=[[1, C]], channel_multiplier=-1,
            )
            g_sb.append(gs)

        # add -1e30 to columns of empty classes (broadcast the row via PE)
        maskp = psmid.tile([P, C], F32, tag="m")
        nc.tensor.matmul(maskp[:], ones_row[:], negmask_r[:])
        for h in range(2):
            nc.vector.tensor_tensor(
                out=g_sb[h][:], in0=g_sb[h][:], in1=maskp[:], op=ALU.add
            )

        # ---- nearest-centroid one-hot (argmax by equality) -------------
        nst = [mid.tile([P, C], F32, tag=f"nst{k}", name=f"nst{k}") for k in range(2)]
        for h in range(2):
            mx = mid.tile([P, 1], F32, tag=f"mx{h}")
            nc.vector.reduce_max(mx[:], g_sb[h][:], axis=mybir.AxisListType.X)
            ns = mid.tile([P, C], F32, tag=f"ns{h}")
            nc.vector.tensor_scalar(
                out=ns[:], in0=g_sb[h][:], scalar1=mx[:], scalar2=None,
                op0=ALU.is_equal,
            )
            for k in range(2):
                tp = psmid.tile([P, P], F32, tag="m")
                nc.tensor.transpose(tp[:], ns[:, k * P : (k + 1) * P], ident[:])
                nc.vector.tensor_copy(
                    out=nst[k][:, h * P : (h + 1) * P], in_=tp[:]
                )

        # ---- u = w*(cent_near - cent), b = w*margin; write table -------
        for h in range(2):
            cnear = psmid.tile([P, D], F32, tag="m")
            for k in range(2):
                nc.tensor.matmul(
                    cnear[:], nst[k][:, h * P : (h + 1) * P], cent[k][:],
                    start=(k == 0), stop=(k == 1),
                )
            # k*u in fp32, then round to the bf16 row; b and k^2|u|^2 are
            # stored as bf16 hi+lo pairs to keep fp32-level precision
            uf = mid.tile([P, D], F32, tag="uf")
            nc.vector.tensor_tensor(
                out=uf[:], in0=cnear[:], in1=cent[h][:], op=ALU.subtract
            )
            nc.vector.tensor_scalar(
                out=uf[:], in0=uf[:], scalar1=wcol[h][:],
                scalar2=KAPPA, op0=ALU.mult, op1=ALU.mult,
            )
            tbl_sb = mid.tile([P, TBL_B], BF16, tag=f"tb{h}")
            nc.vector.tensor_copy(out=tbl_sb[:, 0:D], in_=uf[:])
            bcol = mid.tile([P, 1], F32, tag=f"bc{h}")
            nc.vector.tensor_scalar(
                out=bcol[:], in0=wcol[h][:], scalar1=MARGIN, scalar2=None,
                op0=ALU.mult,
            )
            u2col = mid.tile([P, 1], F32, tag=f"u2{h}")
            squ = sq.tile([P, D], F32, tag="sq")
            nc.scalar.activation(
                squ[:], tbl_sb[:, 0:D], ACTF.Square, accum_out=u2col[:]
            )
            lo = mid.tile([P, 1], F32, tag=f"lo{h}")
            nc.vector.tensor_copy(out=tbl_sb[:, D : D + 1], in_=bcol[:])
            nc.vector.tensor_copy(out=lo[:], in_=tbl_sb[:, D : D + 1])
            nc.vector.tensor_tensor(out=lo[:], in0=bcol[:], in1=lo[:],
                                    op=ALU.subtract)
            nc.vector.tensor_copy(out=tbl_sb[:, D + 1 : D + 2], in_=lo[:])
            nc.vector.tensor_copy(out=tbl_sb[:, D + 2 : D + 3], in_=u2col[:])
            nc.vector.tensor_copy(out=lo[:], in_=tbl_sb[:, D + 2 : D + 3])
            nc.vector.tensor_tensor(out=lo[:], in0=u2col[:], in1=lo[:],
                                    op=ALU.subtract)
            nc.vector.tensor_copy(out=tbl_sb[:, D + 3 : D + 4], in_=lo[:])
            nc.vector.memset(tbl_sb[:, D + 4 : TBL_B], 0.0)
            nc.sync.dma_start(out=table.ap()[h * P : (h + 1) * P, :], in_=tbl_sb[:])

        # ---- pass 2: gather (k*u, b, k^2|u|^2) by label; dot via the ----
        # difference of squares:  e.u = (|e + k*u|^2 - |e|^2 - k^2|u|^2)/2k.
        # (tensor_tensor_reduce is broken on HW; multi-index indirect
        # gathers too — one [P,1]-offset gather per 128-sample tile.)
        q_all = const.tile([P, T], F32)
        bu_all = const.tile([P, T, 4], F32)
        tiles_per_g = GCHUNK // P
        for gc in range(T // tiles_per_g):
            g_t = gat.tile([P, tiles_per_g, TBL_B], BF16, tag="g", name=f"g{gc}")
            nc.gpsimd.dma_gather(
                out_ap=g_t[:], in_ap=table.ap(),
                idxs_ap=lab16_sb[:, gc * (GCHUNK // 16) : (gc + 1) * (GCHUNK // 16)],
                num_idxs=GCHUNK, num_idxs_reg=GCHUNK, elem_size=TBL_B,
            )
            nc.vector.tensor_copy(
                out=bu_all[:, gc * tiles_per_g : (gc + 1) * tiles_per_g, :],
                in_=g_t[:, :, D : D + 4],
            )
            for j in range(tiles_per_g):
                t = gc * tiles_per_g + j
                s_t = sq.tile([P, D], F32, tag="pr")
                nc.vector.tensor_tensor(
                    out=s_t[:], in0=e_tile(t), in1=g_t[:, j, 0:D], op=ALU.add
                )
                sq2 = sq.tile([P, D], F32, tag="sq")
                nc.scalar.activation(
                    sq2[:], s_t[:], ACTF.Square, accum_out=q_all[:, t : t + 1]
                )


        # pre = (q - |e|^2 - k^2|u|^2) * (r / 2k) + b ;  term = relu(pre)
        r2 = const.tile([P, T], F32)
        nc.vector.tensor_scalar(
            out=r2[:], in0=r_all[:], scalar1=1.0 / (2.0 * KAPPA), scalar2=None,
            op0=ALU.mult,
        )
        pre_all = const.tile([P, T], F32)
        nc.vector.tensor_tensor(
            out=pre_all[:], in0=q_all[:], in1=norm2[:], op=ALU.subtract
        )
        nc.vector.tensor_tensor(
            out=pre_all[:], in0=pre_all[:], in1=bu_all[:, :, 2], op=ALU.subtract
        )
        nc.vector.tensor_tensor(
            out=pre_all[:], in0=pre_all[:], in1=bu_all[:, :, 3], op=ALU.subtract
        )
        nc.vector.tensor_tensor(
            out=pre_all[:], in0=pre_all[:], in1=r2[:], op=ALU.mult
        )
        nc.vector.tensor_tensor(
            out=pre_all[:], in0=pre_all[:], in1=bu_all[:, :, 0], op=ALU.add
        )
        nc.vector.tensor_tensor(
            out=pre_all[:], in0=pre_all[:], in1=bu_all[:, :, 1], op=ALU.add
        )
        con_all = const.tile([P, T], F32)
        nc.scalar.activation(con_all[:], pre_all[:], ACTF.Relu)

        tot_col = mid.tile([P, 1], F32, tag="tot")
        nc.vector.reduce_sum(tot_col[:], con_all[:], axis=mybir.AxisListType.X)
        tot_ps = psmid.tile([1, 1], F32, tag="m")
        nc.tensor.matmul(tot_ps[:], tot_col[:], ones_col[:])
        tot_sb = mid.tile([1, 8], F32, tag="totsb")
        nc.vector.memset(tot_sb[:], 0.0)
        nc.vector.tensor_copy(out=tot_sb[:, 0:1], in_=tot_ps[:])
        nc.sync.dma_start(out=ar2_in.ap()[:], in_=tot_sb[:])
        nc.gpsimd.collective_compute(
            "AllReduce", ALU.add, replica_groups=groups,
            ins=[ar2_in.ap()], outs=[ar2_out.ap()],
        )
        gtot = mid.tile([1, 8], F32, tag="gtot")
        nc.sync.dma_start(out=gtot[:], in_=ar2_out.ap()[:])
        loss_sb = mid.tile([1, 1], F32, tag="loss")
        nc.vector.tensor_tensor(
            out=loss_sb[:], in0=gtot[:, 0:1], in1=inv_np[:], op=ALU.mult
        )
        nc.sync.dma_start(out=loss_out.ap()[:], in_=loss_sb[:])

    nc.compile()
    return nc


_NC = None


def _get_nc():
    global _NC
    if _NC is None:
        _NC = _build()
    return _NC


def build_in_maps(emb: np.ndarray, lab: np.ndarray) -> list[dict]:
    """Shard full inputs across the 8 cores (batch-dim data parallel)."""
    in_maps = []
    for c in range(N_CORES):
        sl = slice(c * B_LOC, (c + 1) * B_LOC)
        lab_c = lab[sl]
        lab_2d = np.ascontiguousarray(lab_c.reshape(T, P).T)  # [P, T]
        wrapped = lab_c.astype(np.int16).reshape(B_LOC // 16, 16).T
        lab16_2d = np.ascontiguousarray(np.tile(wrapped, (P // 16, 1)))
        in_maps.append({"emb": emb[sl], "lab": lab_2d, "lab16": lab16_2d})
    return in_maps


def kernel(embeddings: np.ndarray, labels: np.ndarray) -> np.ndarray:
    emb = np.ascontiguousarray(np.asarray(embeddings, dtype=np.float32))
    lab = np.asarray(labels).astype(np.int32)
    assert emb.shape == (B_FULL, D) and lab.shape == (B_FULL,)

    nc = _get_nc()
    in_maps = build_in_maps(emb, lab)
    res = run_bass_kernel_spmd(nc, in_maps, core_ids=list(range(N_CORES)))
    loss = res.results[0]["loss"]
    return np.asarray(loss, dtype=np.float32).reshape(())


if __name__ == "__main__":
    rng = np.random.default_rng(0)
    e = rng.standard_normal((B_FULL, D), dtype=np.float32)
    l = rng.integers(0, C, size=(B_FULL,)).astype(np.int32)
    print(kernel(embeddings=e, labels=l))



# revision 4
# speedup vs baseline: 1.5356x; 1.5356x over previous
"""Centroid triplet loss on 8 Trainium2 NeuronCores (Bass/Tile).

Class-sharded data parallel: the host assigns core k ALL samples whose label
falls in [32k, 32k+32) (padded to a fixed capacity with zero rows + an
out-of-range label).  Per-class embedding sums are then fully core-local, so
the only centroid communication is a 32KB bf16 AllGather of each core's 32
normalized centroid rows (vs. a 514KB AllReduce for unsorted sharding).

Math (equivalent to the reference):
    term_i = relu(margin + r_i * e_i . (cent[near(l_i)] - cent[l_i]))
    loss   = sum_c (1/count_c) * sum_{i in c} term_i / n_present
Per-sample gathers are replaced by matmuls against the 32-class one-hot:
pass 2 computes u_i = onehot_i . U (U = cent_near - cent_own, [32,512]) with
one PE matmul per 128-sample tile, dots it with e_i on DVE, and reduces the
per-class term sums S_c with tiny [128,1]x[128,32] matmuls.  Label-derived
scalars (1/count, presence mask, 1/n_present) are host-computed inputs.
"""

import sys

for _p in ("/opt/trn_rl_repo",):
    if _p not in sys.path:
        sys.path.insert(0, _p)

from contextlib import ExitStack

import ml_dtypes
import numpy as np

from concourse import bacc, bass, mybir, tile
from concourse.bass_utils import run_bass_kernel_spmd
from concourse.masks import make_identity

F32 = mybir.dt.float32
BF16 = mybir.dt.bfloat16
I32 = mybir.dt.int32
ALU = mybir.AluOpType
ACTF = mybir.ActivationFunctionType
AX = mybir.AxisListType.X

N_CORES = 8
B_FULL = 65536
D = 512
C = 256
C_LOC = C // N_CORES        # 32 classes owned per core
MARGIN = 0.3
EPS = 1e-12
NEG = -1e30

P = 128                      # SBUF partitions
B_CAP = 8704                 # padded per-core sample capacity (mean 8192)
T = B_CAP // P               # 68 sample tiles of 128
NCHUNK = 8                   # tiles per norm batch / load DMA
WARMUP_AR = True             # dummy tiny AllReduce to absorb CC bootstrap


def _build():
    nc = bacc.Bacc(
        "TRN2",
        target_bir_lowering=False,
        debug=False,
        enable_asserts=False,
        num_devices=N_CORES,
    )

    emb = nc.dram_tensor("emb", [B_CAP, D], BF16, kind="ExternalInput")
    lab = nc.dram_tensor("lab", [P, T], I32, kind="ExternalInput")
    negmask = nc.dram_tensor("negmask", [C_LOC, C], F32, kind="ExternalInput")
    wrow = nc.dram_tensor("wrow", [1, C_LOC], F32, kind="ExternalInput")
    invnp = nc.dram_tensor("invnp", [1, 1], F32, kind="ExternalInput")
    loss_out = nc.dram_tensor("loss", [1, 1], F32, kind="ExternalOutput")

    ag_in = nc.dram_tensor("ag_in", [C_LOC, D], BF16)
    ag_out = nc.dram_tensor("ag_out", [C, D], BF16, addr_space="Shared")
    ar2_in = nc.dram_tensor("ar2_in", [1, 8], F32)
    ar2_out = nc.dram_tensor("ar2_out", [1, 8], F32, addr_space="Shared")
    if WARMUP_AR:
        ar0_in = nc.dram_tensor("ar0_in", [1, 8], F32)
        ar0_out = nc.dram_tensor("ar0_out", [1, 8], F32, addr_space="Shared")

    groups = [list(range(N_CORES))]

    with tile.TileContext(nc) as tc, ExitStack() as ctx:
        const = ctx.enter_context(tc.tile_pool(name="const", bufs=1))
        big = ctx.enter_context(tc.tile_pool(name="big", bufs=1))
        work = ctx.enter_context(tc.tile_pool(name="work", bufs=3))
        sq = ctx.enter_context(tc.tile_pool(name="sq", bufs=2))
        mid = ctx.enter_context(tc.tile_pool(name="mid", bufs=1))
        psacc = ctx.enter_context(tc.tile_pool(name="psacc", bufs=1, space="PSUM"))
        psmid = ctx.enter_context(tc.tile_pool(name="psmid", bufs=3, space="PSUM"))
        psu = ctx.enter_context(tc.tile_pool(name="psu", bufs=3, space="PSUM"))

        # ---- warm up the collective stream under the load DMA ----------
        if WARMUP_AR:
            ar0_sb = mid.tile([1, 8], F32, tag="ar0")
            nc.vector.memset(ar0_sb[:], 0.0)
            nc.sync.dma_start(out=ar0_in.ap()[:], in_=ar0_sb[:])
            nc.gpsimd.collective_compute(
                "AllReduce", ALU.add, replica_groups=groups,
                ins=[ar0_in.ap()], outs=[ar0_out.ap()],
            )

        # ---- constants -------------------------------------------------
        ident = const.tile([P, P], F32)
        make_identity(nc, ident[:])
        identb = const.tile([P, P], BF16)
        nc.vector.tensor_copy(out=identb[:], in_=ident[:])
        iota_row = const.tile([P, C_LOC], BF16)
        nc.gpsimd.iota(
            iota_row[:], pattern=[[1, C_LOC]], base=0, channel_multiplier=0,
            allow_small_or_imprecise_dtypes=True,
        )
        ones_col = const.tile([P, 1], F32)
        nc.gpsimd.memset(ones_col[:], 1.0)

        lab_sb = const.tile([P, T], I32)
        nc.sync.dma_start(out=lab_sb[:], in_=lab.ap())
        lab_f = const.tile([P, T], BF16)
        nc.vector.tensor_copy(out=lab_f[:], in_=lab_sb[:])
        nm_sb = const.tile([C_LOC, C], F32)
        nc.sync.dma_start(out=nm_sb[:], in_=negmask.ap())
        w_sb = const.tile([1, C_LOC], F32)
        nc.sync.dma_start(out=w_sb[:], in_=wrow.ap())
        invnp_sb = const.tile([1, 1], F32)
        nc.sync.dma_start(out=invnp_sb[:], in_=invnp.ap())

        # ---- pass 1: load embeddings, norms, local class sums ----------
        chunks = []
        t0 = 0
        while t0 < T:
            chunks.append((t0, min(NCHUNK, T - t0)))
            t0 += NCHUNK

        e_chunks = {}
        emb_v = emb.ap().rearrange("(t p) d -> p t d", p=P)
        for (c0, cn) in chunks:
            ec = big.tile([P, cn, D], BF16, tag=f"e{c0}")
            e_chunks[c0] = ec
            nc.gpsimd.dma_start(out=ec[:], in_=emb_v[:, c0 : c0 + cn, :])

        def e_tile(t):
            c0 = (t // NCHUNK) * NCHUNK
            return e_chunks[c0][:, t - c0, :]

        norm2 = const.tile([P, T], F32)
        r_all = const.tile([P, T], F32)
        oht_bf = const.tile([P, T, C_LOC], BF16)   # one-hot, bf16 (pass-1/2 MMs)
        oht_f = const.tile([P, T, C_LOC], F32)     # one-hot, f32 (S matmuls)
        ohtT = const.tile([C_LOC, T * P], BF16)    # transposed one-hot

        sums_ps = psacc.tile([C_LOC, D], F32, tag="sums")

        for (c0, cn) in chunks:
            csl = slice(c0, c0 + cn)
            for j in range(cn):
                t = c0 + j
                et = e_tile(t)
                if t % 2 == 0:
                    sq_t = sq.tile([P, D], F32, tag="sq")
                    nc.scalar.activation(
                        sq_t[:], et, ACTF.Square, accum_out=norm2[:, t : t + 1]
                    )
                else:
                    pr_t = sq.tile([P, D], BF16, tag="pr")
                    nc.vector.tensor_tensor(out=pr_t[:], in0=et, in1=et, op=ALU.mult)
                    nc.vector.reduce_sum(norm2[:, t : t + 1], pr_t[:], axis=AX)
            # batched norm -> r for the chunk (clamped so zero pads stay finite)
            nc.scalar.activation(r_all[:, csl], norm2[:, csl], ACTF.Sqrt)
            nc.vector.tensor_scalar(
                out=r_all[:, csl], in0=r_all[:, csl], scalar1=EPS, scalar2=None,
                op0=ALU.max,
            )
            nc.vector.reciprocal(r_all[:, csl], r_all[:, csl])

            for j in range(cn):
                t = c0 + j
                nc.vector.tensor_tensor(
                    out=oht_bf[:, t, :], in0=iota_row[:],
                    in1=lab_f[:, t : t + 1].to_broadcast([P, C_LOC]),
                    op=ALU.is_equal,
                )
                nc.vector.tensor_copy(out=oht_f[:, t, :], in_=oht_bf[:, t, :])
                osc = work.tile([P, C_LOC], BF16, tag="osc")
                nc.scalar.activation(
                    osc[:], oht_bf[:, t, :], ACTF.Copy,
                    scale=r_all[:, t : t + 1],
                )
                nc.tensor.matmul(
                    sums_ps[:], osc[:], e_tile(t),
                    start=(t == 0), stop=(t == T - 1),
                )

        # ---- local centroids + AllGather -------------------------------
        sums_sb = mid.tile([C_LOC, D], F32, tag="ssb")
        nc.vector.tensor_copy(out=sums_sb[:], in_=sums_ps[:])
        s2 = sq.tile([C_LOC, D], F32, tag="sq")
        cn2 = mid.tile([C_LOC, 1], F32, tag="cn2")
        nc.scalar.activation(s2[:], sums_sb[:], ACTF.Square, accum_out=cn2[:])
        nc.scalar.activation(cn2[:], cn2[:], ACTF.Sqrt)
        nc.vector.tensor_scalar(
            out=cn2[:], in0=cn2[:], scalar1=EPS, scalar2=None, op0=ALU.max
        )
        nc.vector.reciprocal(cn2[:], cn2[:])
        cent_loc = mid.tile([C_LOC, D], F32, tag="centloc")
        nc.vector.tensor_scalar(
            out=cent_loc[:], in0=sums_sb[:], scalar1=cn2[:], scalar2=None,
            op0=ALU.mult,
        )
        cent_bf = mid.tile([C_LOC, D], BF16, tag="centbf")
        nc.vector.tensor_copy(out=cent_bf[:], in_=cent_loc[:])
        nc.sync.dma_start(out=ag_in.ap()[:], in_=cent_bf[:])
        nc.gpsimd.collective_compute(
            "AllGather", ALU.bypass, replica_groups=groups,
            ins=[ag_in.ap()], outs=[ag_out.ap()],
        )

        # one-hot transposes for pass 2 (PE is idle while the AG runs)
        for t in range(T):
            tp = psmid.tile([C_LOC, P], BF16, tag="m")
            nc.tensor.transpose(tp[:], oht_bf[:, t, :], identb[:])
            nc.vector.tensor_copy(
                out=ohtT[:, t * P : (t + 1) * P], in_=tp[:]
            )
        # local centroid transpose (f32 -> bf16 chunks) for the G matmul
        clT = [mid.tile([P, C_LOC], BF16, tag=f"clT{i}", name=f"clT{i}") for i in range(4)]
        for i in range(4):
            tp = psmid.tile([P, C_LOC], F32, tag="m")
            nc.tensor.transpose(
                tp[:], cent_loc[:, i * P : (i + 1) * P], ident[0:C_LOC, 0:C_LOC]
            )
            nc.vector.tensor_copy(out=clT[i][:], in_=tp[:])

        # ---- gathered centroids; G rows; nearest; U --------------------
        cent_all = [mid.tile([P, D], BF16, tag=f"ca{h}", name=f"ca{h}") for h in range(2)]
        for h in range(2):
            nc.sync.dma_start(out=cent_all[h][:], in_=ag_out.ap()[h * P : (h + 1) * P, :])
        centT = [mid.tile([P, C], BF16, tag=f"ct{i}", name=f"ct{i}") for i in range(4)]
        for h in range(2):
            for i in range(4):
                tp = psmid.tile([P, P], BF16, tag="m")
                nc.tensor.transpose(
                    tp[:], cent_all[h][:, i * P : (i + 1) * P], identb[:]
                )
                nc.vector.tensor_copy(
                    out=centT[i][:, h * P : (h + 1) * P], in_=tp[:]
                )

        g_ps = psmid.tile([C_LOC, C], F32, tag="m")
        for i in range(4):
            nc.tensor.matmul(
                g_ps[:], clT[i][:], centT[i][:], start=(i == 0), stop=(i == 3)
            )
        g_sb = mid.tile([C_LOC, C], F32, tag="gsb")
        nc.vector.tensor_tensor(out=g_sb[:], in0=g_ps[:], in1=nm_sb[:], op=ALU.add)
        mx = mid.tile([C_LOC, 1], F32, tag="mx")
        nc.vector.reduce_max(mx[:], g_sb[:], axis=AX)
        ns = mid.tile([C_LOC, C], BF16, tag="ns")
        nc.vector.tensor_scalar(
            out=ns[:], in0=g_sb[:], scalar1=mx[:], scalar2=None, op0=ALU.is_equal
        )
        nsT = [mid.tile([P, C_LOC], BF16, tag=f"nsT{h}", name=f"nsT{h}") for h in range(2)]
        for h in range(2):
            tp = psmid.tile([P, C_LOC], BF16, tag="m")
            nc.tensor.transpose(
                tp[:], ns[:, h * P : (h + 1) * P], identb[0:C_LOC, 0:C_LOC]
            )
            nc.vector.tensor_copy(out=nsT[h][:], in_=tp[:])
        cnear_ps = psmid.tile([C_LOC, D], F32, tag="m")
        for h in range(2):
            nc.tensor.matmul(
                cnear_ps[:], nsT[h][:], cent_all[h][:], start=(h == 0), stop=(h == 1)
            )
        u_sb = mid.tile([C_LOC, D], BF16, tag="usb")
        nc.vector.tensor_tensor(
            out=u_sb[:], in0=cnear_ps[:], in1=cent_loc[:], op=ALU.subtract
        )

        # ---- pass 2: u rows via matmul, dot, relu, per-class sums ------
        dot_all = const.tile([P, T], F32)
        con_all = const.tile([P, T], F32)
        s_ps = psacc.tile([1, C_LOC], F32, tag="Sps")

        for (c0, cn) in chunks:
            for j in range(cn):
                t = c0 + j
                u_ps = psu.tile([P, D], F32, tag="u")
                nc.tensor.matmul(
                    u_ps[:], ohtT[:, t * P : (t + 1) * P], u_sb[:],
                    start=True, stop=True,
                )
                pr_t = sq.tile([P, D], BF16, tag="pr")
                nc.vector.tensor_tensor(
                    out=pr_t[:], in0=e_tile(t), in1=u_ps[:], op=ALU.mult
                )
                if t % 2 == 0:
                    sc_t = sq.tile([P, D], BF16, tag="sq")
                    nc.scalar.activation(
                        sc_t[:], pr_t[:], ACTF.Copy,
                        accum_out=dot_all[:, t : t + 1],
                    )
                else:
                    nc.vector.reduce_sum(dot_all[:, t : t + 1], pr_t[:], axis=AX)
            csl = slice(c0, c0 + cn)
            nc.vector.tensor_tensor(
                out=con_all[:, csl], in0=dot_all[:, csl], in1=r_all[:, csl],
                op=ALU.mult,
            )
            nc.vector.tensor_scalar(
                out=con_all[:, csl], in0=con_all[:, csl], scalar1=float(MARGIN),
                scalar2=None, op0=ALU.add,
            )
            nc.scalar.activation(con_all[:, csl], con_all[:, csl], ACTF.Relu)
            for j in range(cn):
                t = c0 + j
                nc.tensor.matmul(
                    s_ps[:], con_all[:, t : t + 1], oht_f[:, t, :],
                    start=(t == 0), stop=(t == T - 1),
                )

        # ---- loss = sum_c w_c * S_c / n_present (over all cores) -------
        s_sb = mid.tile([1, C_LOC], F32, tag="ssum")
        nc.vector.tensor_tensor(out=s_sb[:], in0=s_ps[:], in1=w_sb[:], op=ALU.mult)
        lloc = mid.tile([1, 1], F32, tag="lloc")
        nc.vector.reduce_sum(lloc[:], s_sb[:], axis=AX)
        tot_sb = mid.tile([1, 8], F32, tag="totsb")
        nc.vector.memset(tot_sb[:], 0.0)
        nc.vector.tensor_copy(out=tot_sb[:, 0:1], in_=lloc[:])
        nc.sync.dma_start(out=ar2_in.ap()[:], in_=tot_sb[:])
        nc.gpsimd.collective_compute(
            "AllReduce", ALU.add, replica_groups=groups,
            ins=[ar2_in.ap()], outs=[ar2_out.ap()],
        )
        gtot = mid.tile([1, 8], F32, tag="gtot")
        nc.sync.dma_start(out=gtot[:], in_=ar2_out.ap()[:])
        loss_sb = mid.tile([1, 1], F32, tag="loss")
        nc.vector.tensor_tensor(
            out=loss_sb[:], in0=gtot[:, 0:1], in1=invnp_sb[:], op=ALU.mult
        )
        nc.sync.dma_start(out=loss_out.ap()[:], in_=loss_sb[:])

    nc.compile()
    return nc


_NC = None


def _get_nc():
    global _NC
    if _NC is None:
        _NC = _build()
    return _NC


def build_in_maps(emb: np.ndarray, lab: np.ndarray) -> list[dict]:
    """Class-shard the full batch: core k owns labels [32k, 32k+32)."""
    counts = np.bincount(lab, minlength=C).astype(np.int64)
    order = np.argsort(lab, kind="stable")
    sorted_lab = lab[order]
    bounds = np.searchsorted(sorted_lab, np.arange(0, C + 1, C_LOC))
    n_present = max(int((counts > 0).sum()), 1)
    inv_np = np.full((1, 1), 1.0 / n_present, np.float32)
    empty_col = counts == 0  # (C,)

    in_maps = []
    for k in range(N_CORES):
        idx = order[bounds[k] : bounds[k + 1]]
        nk = len(idx)
        assert nk <= B_CAP, f"core {k} got {nk} samples > capacity {B_CAP}"
        emb_k = np.zeros((B_CAP, D), dtype=ml_dtypes.bfloat16)
        emb_k[:nk] = emb[idx].astype(ml_dtypes.bfloat16)
        lab_k = np.full((B_CAP,), C_LOC, np.int32)
        lab_k[:nk] = lab[idx] - C_LOC * k
        lab_2d = np.ascontiguousarray(lab_k.reshape(T, P).T)  # [P, T]

        nm = np.where(empty_col[None, :], np.float32(NEG), np.float32(0.0))
        nm = np.tile(nm, (C_LOC, 1)).astype(np.float32)
        rows = np.arange(C_LOC)
        nm[rows, C_LOC * k + rows] = NEG  # self-similarity
        w_k = (
            1.0 / np.maximum(counts[C_LOC * k : C_LOC * (k + 1)], 1)
        ).astype(np.float32)[None, :]

        in_maps.append(
            {
                "emb": emb_k,
                "lab": lab_2d,
                "negmask": np.ascontiguousarray(nm),
                "wrow": np.ascontiguousarray(w_k),
                "invnp": inv_np,
            }
        )
    return in_maps


def kernel(embeddings: np.ndarray, labels: np.ndarray) -> np.ndarray:
    emb = np.ascontiguousarray(np.asarray(embeddings, dtype=np.float32))
    lab = np.asarray(labels).astype(np.int32)
    assert emb.shape == (B_FULL, D) and lab.shape == (B_FULL,)

    nc = _get_nc()
    in_maps = build_in_maps(emb, lab)
    res = run_bass_kernel_spmd(nc, in_maps, core_ids=list(range(N_CORES)))
    loss = res.results[0]["loss"]
    return np.asarray(loss, dtype=np.float32).reshape(())


if __name__ == "__main__":
    rng = np.random.default_rng(0)
    e = rng.standard_normal((B_FULL, D), dtype=np.float32)
    l = rng.integers(0, C, size=(B_FULL,)).astype(np.int32)
    print(kernel(embeddings=e, labels=l))


# revision 7
# speedup vs baseline: 1.7044x; 1.1099x over previous
"""Centroid triplet loss on 8 Trainium2 NeuronCores (Bass/Tile).

Class-sharded data parallel: the host assigns core k ALL samples whose label
falls in [32k, 32k+32) (padded to a fixed capacity with zero rows + an
out-of-range label).  Per-class embedding sums are then fully core-local, so
the only centroid communication is a 32KB bf16 AllGather of each core's 32
normalized centroid rows (vs. a 514KB AllReduce for unsorted sharding).

Math (equivalent to the reference):
    term_i = relu(margin + r_i * e_i . (cent[near(l_i)] - cent[l_i]))
    loss   = sum_c (1/count_c) * sum_{i in c} term_i / n_present
Per-sample gathers are replaced by matmuls against the 32-class one-hot:
pass 2 computes u_i = onehot_i . U (U = cent_near - cent_own, [32,512]) with
one PE matmul per 128-sample tile, dots it with e_i on DVE, and reduces the
per-class term sums S_c with tiny [128,1]x[128,32] matmuls.  Label-derived
scalars (1/count, presence mask, 1/n_present) are host-computed inputs.
"""

import sys

for _p in ("/opt/trn_rl_repo",):
    if _p not in sys.path:
        sys.path.insert(0, _p)

from contextlib import ExitStack

import ml_dtypes
import numpy as np

from concourse import bacc, bass, mybir, tile
from concourse.bass_utils import run_bass_kernel_spmd
from concourse.masks import make_identity

F32 = mybir.dt.float32
BF16 = mybir.dt.bfloat16
I32 = mybir.dt.int32
ALU = mybir.AluOpType
ACTF = mybir.ActivationFunctionType
AX = mybir.AxisListType.X

N_CORES = 8
B_FULL = 65536
D = 512
C = 256
C_LOC = C // N_CORES        # 32 classes owned per core
MARGIN = 0.3
EPS = 1e-12
NEG = -1e30

P = 128                      # SBUF partitions
B_CAP = 8704                 # padded per-core sample capacity (mean 8192)
T = B_CAP // P               # 68 sample tiles of 128
NCHUNK = 8                   # tiles per norm batch / load DMA
WARMUP_AR = True             # dummy tiny AllReduce to absorb CC bootstrap


def _build():
    nc = bacc.Bacc(
        "TRN2",
        target_bir_lowering=False,
        debug=False,
        enable_asserts=False,
        num_devices=N_CORES,
    )

    emb = nc.dram_tensor("emb", [B_CAP, D], BF16, kind="ExternalInput")
    lab = nc.dram_tensor("lab", [P, T], I32, kind="ExternalInput")
    negmask = nc.dram_tensor("negmask", [C_LOC, C], F32, kind="ExternalInput")
    wsamp = nc.dram_tensor("wsamp", [P, T], F32, kind="ExternalInput")
    invnp = nc.dram_tensor("invnp", [1, 1], F32, kind="ExternalInput")
    loss_out = nc.dram_tensor("loss", [1, 1], F32, kind="ExternalOutput")

    ag_in = nc.dram_tensor("ag_in", [C_LOC, D], BF16)
    ag_out = nc.dram_tensor("ag_out", [C, D], BF16, addr_space="Shared")
    ar2_in = nc.dram_tensor("ar2_in", [1, 8], F32)
    ar2_out = nc.dram_tensor("ar2_out", [1, 8], F32, addr_space="Shared")
    if WARMUP_AR:
        ar0_in = nc.dram_tensor("ar0_in", [1, 8], F32)
        ar0_out = nc.dram_tensor("ar0_out", [1, 8], F32, addr_space="Shared")

    groups = [list(range(N_CORES))]

    with tile.TileContext(nc) as tc, ExitStack() as ctx:
        const = ctx.enter_context(tc.tile_pool(name="const", bufs=1))
        big = ctx.enter_context(tc.tile_pool(name="big", bufs=1))
        work = ctx.enter_context(tc.tile_pool(name="work", bufs=3))
        sq = ctx.enter_context(tc.tile_pool(name="sq", bufs=2))
        mid = ctx.enter_context(tc.tile_pool(name="mid", bufs=1))
        psacc = ctx.enter_context(tc.tile_pool(name="psacc", bufs=1, space="PSUM"))
        psmid = ctx.enter_context(tc.tile_pool(name="psmid", bufs=3, space="PSUM"))
        psu = ctx.enter_context(tc.tile_pool(name="psu", bufs=3, space="PSUM"))

        # ---- warm up the collective stream under the load DMA ----------
        if WARMUP_AR:
            ar0_sb = mid.tile([1, 8], F32, tag="ar0")
            nc.vector.memset(ar0_sb[:], 0.0)
            nc.sync.dma_start(out=ar0_in.ap()[:], in_=ar0_sb[:])
            nc.gpsimd.collective_compute(
                "AllReduce", ALU.add, replica_groups=groups,
                ins=[ar0_in.ap()], outs=[ar0_out.ap()],
            )

        # ---- constants -------------------------------------------------
        ident = const.tile([P, P], F32)
        make_identity(nc, ident[:])
        identb = const.tile([P, P], BF16)
        nc.vector.tensor_copy(out=identb[:], in_=ident[:])
        iota_row = const.tile([P, C_LOC], BF16)
        nc.gpsimd.iota(
            iota_row[:], pattern=[[1, C_LOC]], base=0, channel_multiplier=0,
            allow_small_or_imprecise_dtypes=True,
        )
        ones_col = const.tile([P, 1], F32)
        nc.gpsimd.memset(ones_col[:], 1.0)

        lab_sb = const.tile([P, T], I32)
        nc.sync.dma_start(out=lab_sb[:], in_=lab.ap())
        lab_f = const.tile([P, T], BF16)
        nc.vector.tensor_copy(out=lab_f[:], in_=lab_sb[:])
        nm_sb = const.tile([C_LOC, C], F32)
        nc.sync.dma_start(out=nm_sb[:], in_=negmask.ap())
        ws_sb = const.tile([P, T], F32)
        nc.sync.dma_start(out=ws_sb[:], in_=wsamp.ap())
        invnp_sb = const.tile([1, 1], F32)
        nc.sync.dma_start(out=invnp_sb[:], in_=invnp.ap())

        # ---- pass 1: load embeddings, norms, local class sums ----------
        chunks = []
        t0 = 0
        while t0 < T:
            chunks.append((t0, min(NCHUNK, T - t0)))
            t0 += NCHUNK

        e_chunks = {}
        emb_v = emb.ap().rearrange("(t p) d -> p t d", p=P)
        for (c0, cn) in chunks:
            ec = big.tile([P, cn, D], BF16, tag=f"e{c0}")
            e_chunks[c0] = ec
            nc.gpsimd.dma_start(out=ec[:], in_=emb_v[:, c0 : c0 + cn, :])

        def e_tile(t):
            c0 = (t // NCHUNK) * NCHUNK
            return e_chunks[c0][:, t - c0, :]

        norm2 = const.tile([P, T], F32)
        r_all = const.tile([P, T], F32)
        r_bf = const.tile([P, T], BF16)
        oht_bf = const.tile([P, T, C_LOC], BF16)   # one-hot, bf16 (pass-1/2 MMs)
        ohtT = const.tile([C_LOC, T * P], BF16)    # transposed one-hot

        sums_ps = psacc.tile([C_LOC, D], F32, tag="sums")

        for (c0, cn) in chunks:
            csl = slice(c0, c0 + cn)
            for j in range(cn):
                t = c0 + j
                et = e_tile(t)
                if t % 3 != 2:
                    sq_t = sq.tile([P, D], F32, tag="sq")
                    nc.scalar.activation(
                        sq_t[:], et, ACTF.Square, accum_out=norm2[:, t : t + 1]
                    )
                else:
                    pr_t = sq.tile([P, D], BF16, tag="pr")
                    nc.gpsimd.tensor_tensor(out=pr_t[:], in0=et, in1=et, op=ALU.mult)
                    nc.vector.reduce_sum(norm2[:, t : t + 1], pr_t[:], axis=AX)
            # batched norm -> r for the chunk (clamped so zero pads stay finite)
            nc.scalar.activation(r_all[:, csl], norm2[:, csl], ACTF.Sqrt)
            nc.vector.tensor_scalar(
                out=r_all[:, csl], in0=r_all[:, csl], scalar1=EPS, scalar2=None,
                op0=ALU.max,
            )
            nc.vector.reciprocal(r_all[:, csl], r_all[:, csl])
            nc.vector.tensor_copy(out=r_bf[:, csl], in_=r_all[:, csl])

            for j in range(cn):
                t = c0 + j
                nc.vector.tensor_tensor(
                    out=oht_bf[:, t, :], in0=iota_row[:],
                    in1=lab_f[:, t : t + 1].to_broadcast([P, C_LOC]),
                    op=ALU.is_equal,
                )
                osc = work.tile([P, C_LOC], BF16, tag="osc")
                nc.vector.tensor_tensor(
                    out=osc[:], in0=oht_bf[:, t, :],
                    in1=r_bf[:, t : t + 1].to_broadcast([P, C_LOC]),
                    op=ALU.mult,
                )
                nc.tensor.matmul(
                    sums_ps[:], osc[:], e_tile(t),
                    start=(t == 0), stop=(t == T - 1),
                )

        # ---- local centroids + AllGather -------------------------------
        sums_sb = mid.tile([C_LOC, D], F32, tag="ssb")
        nc.vector.tensor_copy(out=sums_sb[:], in_=sums_ps[:])
        s2 = sq.tile([C_LOC, D], F32, tag="sq")
        cn2 = mid.tile([C_LOC, 1], F32, tag="cn2")
        nc.scalar.activation(s2[:], sums_sb[:], ACTF.Square, accum_out=cn2[:])
        nc.scalar.activation(cn2[:], cn2[:], ACTF.Sqrt)
        nc.vector.tensor_scalar(
            out=cn2[:], in0=cn2[:], scalar1=EPS, scalar2=None, op0=ALU.max
        )
        nc.vector.reciprocal(cn2[:], cn2[:])
        cent_loc = mid.tile([C_LOC, D], F32, tag="centloc")
        nc.vector.tensor_scalar(
            out=cent_loc[:], in0=sums_sb[:], scalar1=cn2[:], scalar2=None,
            op0=ALU.mult,
        )
        cent_bf = mid.tile([C_LOC, D], BF16, tag="centbf")
        nc.vector.tensor_copy(out=cent_bf[:], in_=cent_loc[:])
        nc.sync.dma_start(out=ag_in.ap()[:], in_=cent_bf[:])
        nc.gpsimd.collective_compute(
            "AllGather", ALU.bypass, replica_groups=groups,
            ins=[ag_in.ap()], outs=[ag_out.ap()],
        )

        # one-hot transposes for pass 2 (PE is idle while the AG runs)
        for t in range(T):
            tp = psmid.tile([C_LOC, P], BF16, tag="m")
            nc.tensor.transpose(tp[:], oht_bf[:, t, :], identb[:])
            nc.vector.tensor_copy(
                out=ohtT[:, t * P : (t + 1) * P], in_=tp[:]
            )
        # local centroid transpose (f32 -> bf16 chunks) for the G matmul
        clT = [mid.tile([P, C_LOC], BF16, tag=f"clT{i}", name=f"clT{i}") for i in range(4)]
        for i in range(4):
            tp = psmid.tile([P, C_LOC], F32, tag="m")
            nc.tensor.transpose(
                tp[:], cent_loc[:, i * P : (i + 1) * P], ident[0:C_LOC, 0:C_LOC]
            )
            nc.vector.tensor_copy(out=clT[i][:], in_=tp[:])

        # ---- gathered centroids; G rows; nearest; U --------------------
        cent_all = [mid.tile([P, D], BF16, tag=f"ca{h}", name=f"ca{h}") for h in range(2)]
        for h in range(2):
            nc.sync.dma_start(out=cent_all[h][:], in_=ag_out.ap()[h * P : (h + 1) * P, :])
        centT = [mid.tile([P, C], BF16, tag=f"ct{i}", name=f"ct{i}") for i in range(4)]
        for h in range(2):
            for i in range(4):
                tp = psmid.tile([P, P], BF16, tag="m")
                nc.tensor.transpose(
                    tp[:], cent_all[h][:, i * P : (i + 1) * P], identb[:]
                )
                nc.vector.tensor_copy(
                    out=centT[i][:, h * P : (h + 1) * P], in_=tp[:]
                )

        g_ps = psmid.tile([C_LOC, C], F32, tag="m")
        for i in range(4):
            nc.tensor.matmul(
                g_ps[:], clT[i][:], centT[i][:], start=(i == 0), stop=(i == 3)
            )
        g_sb = mid.tile([C_LOC, C], F32, tag="gsb")
        nc.vector.tensor_tensor(out=g_sb[:], in0=g_ps[:], in1=nm_sb[:], op=ALU.add)
        mx = mid.tile([C_LOC, 1], F32, tag="mx")
        nc.vector.reduce_max(mx[:], g_sb[:], axis=AX)
        ns = mid.tile([C_LOC, C], BF16, tag="ns")
        nc.vector.tensor_scalar(
            out=ns[:], in0=g_sb[:], scalar1=mx[:], scalar2=None, op0=ALU.is_equal
        )
        nsT = [mid.tile([P, C_LOC], BF16, tag=f"nsT{h}", name=f"nsT{h}") for h in range(2)]
        for h in range(2):
            tp = psmid.tile([P, C_LOC], BF16, tag="m")
            nc.tensor.transpose(
                tp[:], ns[:, h * P : (h + 1) * P], identb[0:C_LOC, 0:C_LOC]
            )
            nc.vector.tensor_copy(out=nsT[h][:], in_=tp[:])
        cnear_ps = psmid.tile([C_LOC, D], F32, tag="m")
        for h in range(2):
            nc.tensor.matmul(
                cnear_ps[:], nsT[h][:], cent_all[h][:], start=(h == 0), stop=(h == 1)
            )
        u_sb = mid.tile([C_LOC, D], BF16, tag="usb")
        nc.vector.tensor_tensor(
            out=u_sb[:], in0=cnear_ps[:], in1=cent_loc[:], op=ALU.subtract
        )

        # ---- pass 2: u rows via matmul, dot, relu, weighted sum --------
        dot_all = const.tile([P, T], F32)
        con_all = const.tile([P, T], F32)

        for (c0, cn) in chunks:
            for j in range(cn):
                t = c0 + j
                u_ps = psu.tile([P, D], F32, tag="u")
                nc.tensor.matmul(
                    u_ps[:], ohtT[:, t * P : (t + 1) * P], u_sb[:],
                    start=True, stop=True,
                )
                pr_t = sq.tile([P, D], BF16, tag="pr")
                nc.vector.tensor_tensor(
                    out=pr_t[:], in0=e_tile(t), in1=u_ps[:], op=ALU.mult
                )
                if t % 3 != 2:
                    sc_t = sq.tile([P, D], BF16, tag="sq")
                    nc.scalar.activation(
                        sc_t[:], pr_t[:], ACTF.Copy,
                        accum_out=dot_all[:, t : t + 1],
                    )
                else:
                    nc.vector.reduce_sum(dot_all[:, t : t + 1], pr_t[:], axis=AX)
            csl = slice(c0, c0 + cn)
            nc.vector.tensor_tensor(
                out=con_all[:, csl], in0=dot_all[:, csl], in1=r_all[:, csl],
                op=ALU.mult,
            )
            nc.vector.tensor_scalar(
                out=con_all[:, csl], in0=con_all[:, csl], scalar1=float(MARGIN),
                scalar2=None, op0=ALU.add,
            )
            nc.scalar.activation(con_all[:, csl], con_all[:, csl], ACTF.Relu)

        # ---- loss = sum_i w_i * term_i / n_present (over all cores) ----
        nc.vector.tensor_tensor(
            out=con_all[:], in0=con_all[:], in1=ws_sb[:], op=ALU.mult
        )
        tot_col = mid.tile([P, 1], F32, tag="tot")
        nc.vector.reduce_sum(tot_col[:], con_all[:], axis=AX)
        tot_ps = psmid.tile([1, 1], F32, tag="m")
        nc.tensor.matmul(tot_ps[:], tot_col[:], ones_col[:])
        lloc = mid.tile([1, 1], F32, tag="lloc")
        nc.vector.tensor_copy(out=lloc[:], in_=tot_ps[:])
        tot_sb = mid.tile([1, 8], F32, tag="totsb")
        nc.vector.memset(tot_sb[:], 0.0)
        nc.vector.tensor_copy(out=tot_sb[:, 0:1], in_=lloc[:])
        nc.sync.dma_start(out=ar2_in.ap()[:], in_=tot_sb[:])
        nc.gpsimd.collective_compute(
            "AllReduce", ALU.add, replica_groups=groups,
            ins=[ar2_in.ap()], outs=[ar2_out.ap()],
        )
        gtot = mid.tile([1, 8], F32, tag="gtot")
        nc.sync.dma_start(out=gtot[:], in_=ar2_out.ap()[:])
        loss_sb = mid.tile([1, 1], F32, tag="loss")
        nc.vector.tensor_tensor(
            out=loss_sb[:], in0=gtot[:, 0:1], in1=invnp_sb[:], op=ALU.mult
        )
        nc.sync.dma_start(out=loss_out.ap()[:], in_=loss_sb[:])

    nc.compile()
    return nc


_NC = None


def _get_nc():
    global _NC
    if _NC is None:
        _NC = _build()
    return _NC


def build_in_maps(emb: np.ndarray, lab: np.ndarray) -> list[dict]:
    """Class-shard the full batch: core k owns labels [32k, 32k+32)."""
    counts = np.bincount(lab, minlength=C).astype(np.int64)
    order = np.argsort(lab, kind="stable")
    sorted_lab = lab[order]
    bounds = np.searchsorted(sorted_lab, np.arange(0, C + 1, C_LOC))
    n_present = max(int((counts > 0).sum()), 1)
    inv_np = np.full((1, 1), 1.0 / n_present, np.float32)
    empty_col = counts == 0  # (C,)

    in_maps = []
    for k in range(N_CORES):
        idx = order[bounds[k] : bounds[k + 1]]
        nk = len(idx)
        assert nk <= B_CAP, f"core {k} got {nk} samples > capacity {B_CAP}"
        emb_k = np.zeros((B_CAP, D), dtype=ml_dtypes.bfloat16)
        emb_k[:nk] = emb[idx].astype(ml_dtypes.bfloat16)
        lab_k = np.full((B_CAP,), C_LOC, np.int32)
        lab_k[:nk] = lab[idx] - C_LOC * k
        lab_2d = np.ascontiguousarray(lab_k.reshape(T, P).T)  # [P, T]

        nm = np.where(empty_col[None, :], np.float32(NEG), np.float32(0.0))
        nm = np.tile(nm, (C_LOC, 1)).astype(np.float32)
        rows = np.arange(C_LOC)
        nm[rows, C_LOC * k + rows] = NEG  # self-similarity
        w33 = np.zeros(C_LOC + 1, np.float32)
        w33[:C_LOC] = 1.0 / np.maximum(counts[C_LOC * k : C_LOC * (k + 1)], 1)
        ws_k = np.ascontiguousarray(w33[lab_k].reshape(T, P).T)  # [P, T]

        in_maps.append(
            {
                "emb": emb_k,
                "lab": lab_2d,
                "negmask": np.ascontiguousarray(nm),
                "wsamp": ws_k,
                "invnp": inv_np,
            }
        )
    return in_maps


def kernel(embeddings: np.ndarray, labels: np.ndarray) -> np.ndarray:
    emb = np.ascontiguousarray(np.asarray(embeddings, dtype=np.float32))
    lab = np.asarray(labels).astype(np.int32)
    assert emb.shape == (B_FULL, D) and lab.shape == (B_FULL,)

    nc = _get_nc()
    in_maps = build_in_maps(emb, lab)
    res = run_bass_kernel_spmd(nc, in_maps, core_ids=list(range(N_CORES)))
    loss = res.results[0]["loss"]
    return np.asarray(loss, dtype=np.float32).reshape(())


if __name__ == "__main__":
    rng = np.random.default_rng(0)
    e = rng.standard_normal((B_FULL, D), dtype=np.float32)
    l = rng.integers(0, C, size=(B_FULL,)).astype(np.int32)
    print(kernel(embeddings=e, labels=l))
